# revision 1
# baseline (speedup 1.0000x reference)
"""TRN2 Bass kernel for nn_ClusteringLayer (vq_codebook).

Computes, for inputs x (131072, 256) and clusters c (256, 256):
    dist2[r,k] = ||x_r||^2 + ||c_k||^2 - 2 x_r.c_k
    q = 1/(1+dist2);  q = q / sum_k q          (ALPHA=1 -> power is a no-op)

Strategy (data-parallel over 8 NeuronCores, 16384 rows each):
  - All matmul operands are fp16 (PE runs 1 cycle/row vs 2 for fp32r, and
    round-to-nearest fp16 beats the hardware's truncating fp32r numerically).
    x2 and c2 are computed from the SAME rounded operands, so the distance
    is consistent; total error ~1.5e-4 relative.
  - Host prep: x -> fp16, transposed to [d, r] (contraction on partitions);
    cluster table -> -2*fp16(c); c2 from fp16(c).
  - Device, per 1024-row supertile: p[r,k] = 1 + dist2 accumulated in PSUM:
      2 fp16 main matmuls (xT chunk stationary) x (-2c chunk moving)
      + k=2 aug matmul {x2-256, 1} x {1s, c2-256}   (values small in fp16)
      + k=1 const matmul {1s} x {513s}              (513 exact in fp16)
    x2 row vector: ACT square (fp16) + ones-weighted PE reduce; the
    {x2-256, 1} aug rows come from one ACT copy with bias [-256, 1].
    Then q_un = reciprocal_approx_fast(p) on DVE (one op per 4-block PSUM
    tile), row-sums via one grouped DVE reduce, out = q_un / s via GPSIMD
    normalize_recip.
  - Matmult instructions can carry only ONE sync-wait: all PE-side consts
    live in one DMA'd tile fenced by one dummy matmul; each supertile's xt
    DMA is fenced the same way.
"""

import os
import sys

for _p in ("/root/.axon_site/_ro/trn_rl_repo", "/opt/trn_rl_repo"):
    if os.path.isdir(_p) and _p not in sys.path:
        sys.path.append(_p)

import numpy as np

from concourse import bacc, tile
import concourse.mybir as mybir
from concourse.bass_utils import run_bass_kernel_spmd

F32 = mybir.dt.float32
F16 = mybir.dt.float16

NCORES = 8
B = 131072
D = 256
K = 256
R = B // NCORES          # rows per core
S = 1024                 # rows per supertile
NB = S // 128            # 128-row blocks per supertile
NST = R // S             # supertiles per core
GSZ = 512                # x2-reduce matmul free-dim group size
NG = S // GSZ
BPP = 4                  # blocks per PSUM tile ([128, BPP*256] = 2 banks)

# konst tile column layout (fp16, [128, KW]):
#   [0:512)    ct: -2*c.T chunks; ct[p, ch*256+k] = -2*fp16(c)[k, ch*128+p]
#   [512:768)  caug3 rows {1s, c2-256, 513s}  (rows 0-2; 513 exact in fp16)
KW = 768
KO_CT, KO_CAUG = 0, 512
WARMUP_MMS = 32

_nc_cache = None


def _build():
    nc = bacc.Bacc("TRN2", target_bir_lowering=False, debug=False,
                   num_devices=NCORES)
    xt_d = nc.dram_tensor("xt", [128, 2, R], F16, kind="ExternalInput").ap()
    x2_d = nc.dram_tensor("x2", [1, R], F16, kind="ExternalInput").ap()
    ko_d = nc.dram_tensor("ko", [128, KW], F16, kind="ExternalInput").ap()
    out_d = nc.dram_tensor("out", [R, K], F32, kind="ExternalOutput").ap()

    with tile.TileContext(nc) as tc:
        with (
            tc.tile_pool(name="const", bufs=1) as cpool,
            tc.tile_pool(name="xtp", bufs=6) as xtpool,
            tc.tile_pool(name="qunp", bufs=3) as qunpool,
            tc.tile_pool(name="outp", bufs=3) as outpool,
            tc.tile_pool(name="augp", bufs=6) as augpool,
            tc.tile_pool(name="sp", bufs=4) as spool,
            tc.tile_pool(name="qps", bufs=3, space="PSUM") as qpool,
            tc.tile_pool(name="x2ps", bufs=1, space="PSUM") as x2pool,
        ):
            ko_t = cpool.tile([128, KW], F16, tag="ko")
            nc.sync.dma_start(ko_t[:], ko_d[:])

            ct = ko_t[:, KO_CT:KO_CT + 512].rearrange("p (c k) -> p c k", c=2)
            caug3 = ko_t[0:3, KO_CAUG:KO_CAUG + K]

            # Prologue: one fence matmul absorbs the konst DMA wait, then a
            # dense burst of dummy matmuls to un-throttle the PE clock (HAM).
            fence_p = x2pool.tile([1, GSZ], F32, tag="x2p")
            nc.tensor.matmul(fence_p[0:1, 0:8], ko_t[:, 0:1], ko_t[:, 0:8],
                             start=True, stop=True)
            for _ in range(WARMUP_MMS):
                nc.tensor.matmul(fence_p[:], ko_t[:, 0:1], ko_t[:, 0:GSZ],
                                 start=True, stop=True)

            # aug lhsT tiles [3, S]: row0 = x2-256 (per supertile), rows 1-2
            # stay 1.0 forever. Init whole tile via ACT (single producer
            # engine; direct writes to rows 1-2 would be partition-illegal).
            aug_tiles = []
            for _ in range(6):
                a = augpool.tile([3, S], F16, tag="aug")
                for j in range(NG):
                    nc.scalar.activation(
                        a[:, j * GSZ:(j + 1) * GSZ], ko_t[0:3, 0:GSZ],
                        mybir.ActivationFunctionType.Identity,
                        bias=1.0, scale=0.0,
                    )
                aug_tiles.append(a)

            for st in range(NST):
                r0 = st * S
                xt_t = xtpool.tile([128, 2, S], F16, tag="xt")
                nc.sync.dma_start(xt_t[:], xt_d[:, :, r0:r0 + S])
                aug_t = aug_tiles[st % 6]
                # aug row0 = x2 - 256 arrives by DMA (rows 1-2 remain 1.0)
                nc.sync.dma_start(aug_t[0:1, :], x2_d[0:1, r0:r0 + S])

                if st % 2 == 1:
                    # re-warm burst: keeps the HAM clock gate at K=8/8
                    for _ in range(4):
                        nc.tensor.matmul(fence_p[:], ko_t[:, 0:1],
                                         ko_t[:, 0:GSZ],
                                         start=True, stop=True)

                # per-supertile fence absorbs the xt DMA wait
                nc.tensor.matmul(fence_p[0:1, 0:8], xt_t[:, 0, 0:1],
                                 xt_t[:, 0, 0:8], start=True, stop=True)

                qun_t = qunpool.tile([128, NB, K], F32, tag="qun")
                s_t = spool.tile([128, NB], F32, tag="s")
                for bp in range(NB // BPP):
                    # BPP 128-row blocks share one 2-bank [128, BPP*256] psum
                    qp = qpool.tile([128, BPP, K], F32, tag="qp")
                    for h in range(BPP):
                        b = BPP * bp + h
                        for ch in range(2):
                            nc.tensor.matmul(
                                qp[:, h, :],
                                xt_t[:, ch, b * 128:(b + 1) * 128],
                                ct[:, ch, :],
                                start=(ch == 0), stop=False,
                            )
                        nc.tensor.matmul(
                            qp[:, h, :],
                            aug_t[:, b * 128:(b + 1) * 128],
                            caug3,
                            start=False, stop=True,
                        )
                    nc.vector.reciprocal_approx_fast(
                        out=qun_t[:, BPP * bp:BPP * (bp + 1), :], in_=qp[:])
                    nc.vector.tensor_reduce(
                        s_t[:, BPP * bp:BPP * (bp + 1)],
                        qun_t[:, BPP * bp:BPP * (bp + 1), :],
                        axis=mybir.AxisListType.X, op=mybir.AluOpType.add,
                    )

                out_t = outpool.tile([128, NB, K], F32, tag="out")
                for b in range(NB):
                    nc.gpsimd.normalize_recip(
                        out_t[:, b, :], qun_t[:, b, :], s_t[:, b:b + 1])

                # two half-supertile output DMAs: the first only waits on
                # normalize of blocks 0-3, shortening the drain tail
                half = S // 2
                for hh in range(2):
                    nc.sync.dma_start(
                        out_d[r0 + hh * half:r0 + (hh + 1) * half, :]
                        .rearrange("(b p) k -> p b k", p=128),
                        out_t[:, hh * (NB // 2):(hh + 1) * (NB // 2), :],
                    )
    nc.compile()
    return nc


def _get_nc():
    global _nc_cache
    if _nc_cache is None:
        _nc_cache = _build()
    return _nc_cache


def _prep_in_maps(inputs, clusters):
    x = np.asarray(inputs, dtype=np.float32)
    c = np.asarray(clusters, dtype=np.float32)

    xh = x.astype(np.float16)
    # [core][p, ch, r] = xh[core*R + r, ch*128 + p]
    xt_all = np.ascontiguousarray(
        xh.reshape(NCORES, R, 2, 128).transpose(0, 3, 2, 1))
    # x2 row (consistent with the fp16-rounded x), shifted by -256
    x2_all = ((xh.astype(np.float64) ** 2).sum(1) - 256.0).astype(
        np.float16).reshape(NCORES, 1, R)

    ch = c.astype(np.float16)
    c2h = (ch.astype(np.float64) ** 2).sum(1)

    ko = np.zeros((128, KW), np.float16)
    # ct: -2 * ch.T  (exact doubling in fp16)
    ko[:, KO_CT:KO_CT + 512] = np.ascontiguousarray(
        (-2.0 * ch.astype(np.float32)).astype(np.float16).T
    ).reshape(2, 128, K).transpose(1, 0, 2).reshape(128, 512)
    ko[0, KO_CAUG:KO_CAUG + K] = 1.0
    ko[1, KO_CAUG:KO_CAUG + K] = (c2h - 256.0).astype(np.float16)
    ko[2, KO_CAUG:KO_CAUG + K] = 513.0

    return [
        {"xt": xt_all[i], "x2": x2_all[i], "ko": ko}
        for i in range(NCORES)
    ]


def _run(inputs, clusters, trace=False, tmpdir=None):
    nc = _get_nc()
    in_maps = _prep_in_maps(inputs, clusters)
    res = run_bass_kernel_spmd(nc, in_maps, list(range(NCORES)),
                               trace=trace, tmpdir=tmpdir)
    out = np.concatenate([res.results[i]["out"] for i in range(NCORES)], axis=0)
    return out, res


def kernel(inputs, clusters):
    out, _ = _run(inputs, clusters, trace=False)
    return out



# revision 2
# speedup vs baseline: 1.1744x; 1.1744x over previous
"""TRN2 Bass kernel for nn_ClusteringLayer (vq_codebook).

Computes, for inputs x (131072, 256) and clusters c (256, 256):
    dist2[r,k] = ||x_r||^2 + ||c_k||^2 - 2 x_r.c_k
    q = 1/(1+dist2);  q = q / sum_k q          (ALPHA=1 -> power is a no-op)

Strategy (data-parallel over 8 NeuronCores, 16384 rows each):
  - All matmul operands are fp16 (PE runs 1 cycle/row vs 2 for fp32r, and
    round-to-nearest fp16 beats the hardware's truncating fp32r numerically).
    x2 and c2 are computed from the SAME rounded operands, so the distance
    is consistent.
  - Host prep: x -> fp16, laid out [p, st, ch, s] so each supertile DMA is a
    4KB-per-partition contiguous read; cluster table -> -2*fp16(c); c2 from
    fp16(c).
  - Device, per 1024-row supertile: p[r,k] = 1 + dist2 accumulated in PSUM:
      2 fp16 main matmuls (xT chunk stationary) x (-2c chunk moving)
      + k=3 aug matmul {x2-256, 1, 1} x {1s, c2-256, 513s}
    x2 aug row arrives by DMA (host-computed from the fp16-rounded x).
  - Elementwise work is spread across three engines so none exceeds ~48us:
      DVE:    reciprocal_approx_fast PSUM->SBUF (whole), block-7 row-sum,
              block-7 normalize (tensor_scalar by 1/s)
      ACT:    blocks 0-6 row-sums via activation(Copy, accum_out=s)
      GPSIMD: blocks 0-6 normalize via normalize_recip (fp32 in, fp16 out)
  - Output is written fp16 (q in [0,1]; fp16 rounding ~5e-4 rel, far inside
    the 2e-2 gate), DRAM layout [p, st, b, k] so each half-supertile store is
    a 2KB-per-partition contiguous run; the host unscrambles to [r, k] fp32.
  - Matmult instructions can carry only ONE sync-wait: all PE-side consts
    live in one DMA'd tile fenced by one dummy matmul; each supertile's xt
    DMA is fenced the same way. Dummy-matmul bursts keep the HAM clock gate
    at K=8/8.
"""

import os
import sys

for _p in ("/root/.axon_site/_ro/trn_rl_repo", "/opt/trn_rl_repo"):
    if os.path.isdir(_p) and _p not in sys.path:
        sys.path.append(_p)

import numpy as np

from concourse import bacc, tile
import concourse.mybir as mybir
from concourse.bass_utils import run_bass_kernel_spmd

F32 = mybir.dt.float32
F16 = mybir.dt.float16

NCORES = 8
B = 131072
D = 256
K = 256
R = B // NCORES          # rows per core
S = 1024                 # rows per supertile
NB = S // 128            # 128-row blocks per supertile
NST = R // S             # supertiles per core
GSZ = 512                # warmup matmul free-dim size
BPP = 4                  # blocks per PSUM tile ([128, BPP*256] = 2 banks)

# konst tile column layout (fp16, [128, KW]):
#   [0:512)    ct: -2*c.T chunks; ct[p, ch*256+k] = -2*fp16(c)[k, ch*128+p]
#   [512:768)  caug3 rows {1s, c2-256, 513s}  (rows 0-2; 513 exact in fp16)
KW = 768
KO_CT, KO_CAUG = 0, 512
WARMUP_MMS = 32

_nc_cache = None


def _build():
    nc = bacc.Bacc("TRN2", target_bir_lowering=False, debug=False,
                   num_devices=NCORES)
    xt_d = nc.dram_tensor("xt", [128, NST, 2, S], F16, kind="ExternalInput").ap()
    x2_d = nc.dram_tensor("x2", [1, R], F16, kind="ExternalInput").ap()
    ko_d = nc.dram_tensor("ko", [128, KW], F16, kind="ExternalInput").ap()
    out_d = nc.dram_tensor("out", [128, NST, NB, K], F16,
                           kind="ExternalOutput").ap()

    with tile.TileContext(nc) as tc:
        with (
            tc.tile_pool(name="const", bufs=1) as cpool,
            tc.tile_pool(name="xtp", bufs=6) as xtpool,
            tc.tile_pool(name="qunp", bufs=3) as qunpool,
            tc.tile_pool(name="o16p", bufs=3) as o16pool,
            tc.tile_pool(name="augp", bufs=6) as augpool,
            tc.tile_pool(name="sp", bufs=4) as spool,
            tc.tile_pool(name="rsp", bufs=4) as rspool,
            tc.tile_pool(name="qps", bufs=3, space="PSUM") as qpool,
            tc.tile_pool(name="x2ps", bufs=1, space="PSUM") as x2pool,
        ):
            ko_t = cpool.tile([128, KW], F16, tag="ko")
            nc.sync.dma_start(ko_t[:], ko_d[:])
            # ACT's dummy copy destination (per-supertile row-sum side
            # output); ACT is serial so all copies may share one scratch.
            act_scr = cpool.tile([128, K], F16, tag="actscr")

            ct = ko_t[:, KO_CT:KO_CT + 512].rearrange("p (c k) -> p c k", c=2)
            caug3 = ko_t[0:3, KO_CAUG:KO_CAUG + K]

            # Prologue: one fence matmul absorbs the konst DMA wait, then a
            # dense burst of dummy matmuls to un-throttle the PE clock (HAM).
            fence_p = x2pool.tile([1, GSZ], F32, tag="x2p")
            nc.tensor.matmul(fence_p[0:1, 0:8], ko_t[:, 0:1], ko_t[:, 0:8],
                             start=True, stop=True)
            for _ in range(WARMUP_MMS):
                nc.tensor.matmul(fence_p[:], ko_t[:, 0:1], ko_t[:, 0:GSZ],
                                 start=True, stop=True)

            # aug lhsT tiles [3, S]: row0 = x2-256 (per supertile), rows 1-2
            # stay 1.0 forever. Init whole tile via ACT (single producer
            # engine; direct writes to rows 1-2 would be partition-illegal).
            aug_tiles = []
            for _ in range(6):
                a = augpool.tile([3, S], F16, tag="aug")
                for j in range(S // GSZ):
                    nc.scalar.activation(
                        a[:, j * GSZ:(j + 1) * GSZ], ko_t[0:3, 0:GSZ],
                        mybir.ActivationFunctionType.Identity,
                        bias=1.0, scale=0.0,
                    )
                aug_tiles.append(a)

            for st in range(NST):
                xt_t = xtpool.tile([128, 2, S], F16, tag="xt")
                nc.sync.dma_start(xt_t[:], xt_d[:, st])
                aug_t = aug_tiles[st % 6]
                # aug row0 = x2 - 256 arrives by DMA (rows 1-2 remain 1.0)
                nc.sync.dma_start(aug_t[0:1, :], x2_d[0:1, st * S:(st + 1) * S])

                if st % 2 == 1:
                    # re-warm burst: keeps the HAM clock gate at K=8/8
                    for _ in range(4):
                        nc.tensor.matmul(fence_p[:], ko_t[:, 0:1],
                                         ko_t[:, 0:GSZ],
                                         start=True, stop=True)

                # per-supertile fence absorbs the xt DMA wait
                nc.tensor.matmul(fence_p[0:1, 0:8], xt_t[:, 0, 0:1],
                                 xt_t[:, 0, 0:8], start=True, stop=True)

                qun_t = qunpool.tile([128, NB, K], F32, tag="qun")
                o16_t = o16pool.tile([128, NB, K], F16, tag="o16")
                s_t = spool.tile([128, NB], F32, tag="s")
                rs_t = rspool.tile([128, 1], F32, tag="rs")
                for bp in range(NB // BPP):
                    # BPP 128-row blocks share one 2-bank [128, BPP*256] psum
                    qp = qpool.tile([128, BPP, K], F32, tag="qp")
                    for h in range(BPP):
                        b = BPP * bp + h
                        for ch in range(2):
                            nc.tensor.matmul(
                                qp[:, h, :],
                                xt_t[:, ch, b * 128:(b + 1) * 128],
                                ct[:, ch, :],
                                start=(ch == 0), stop=False,
                            )
                        nc.tensor.matmul(
                            qp[:, h, :],
                            aug_t[:, b * 128:(b + 1) * 128],
                            caug3,
                            start=False, stop=True,
                        )
                    nc.vector.reciprocal_approx_fast(
                        out=qun_t[:, BPP * bp:BPP * (bp + 1), :], in_=qp[:])

                # Row-sums: blocks 0-6 on ACT (fused into a throwaway fp16
                # copy), block 7 on DVE.
                for b in range(NB - 1):
                    nc.scalar.activation(
                        act_scr[:], qun_t[:, b, :],
                        mybir.ActivationFunctionType.Copy,
                        accum_out=s_t[:, b:b + 1],
                    )
                nc.vector.tensor_reduce(
                    s_t[:, NB - 1:NB], qun_t[:, NB - 1, :],
                    axis=mybir.AxisListType.X, op=mybir.AluOpType.add,
                )
                nc.vector.reciprocal_approx_fast(
                    out=rs_t[:], in_=s_t[:, NB - 1:NB])

                # Normalize: blocks 0-6 on GPSIMD, block 7 on DVE.
                for b in range(NB - 1):
                    nc.gpsimd.normalize_recip(
                        o16_t[:, b, :], qun_t[:, b, :], s_t[:, b:b + 1])
                nc.vector.tensor_scalar_mul(
                    o16_t[:, NB - 1, :], qun_t[:, NB - 1, :], rs_t[:, 0:1])

                # two half-supertile output DMAs: the first only waits on
                # normalize of blocks 0-3, shortening the drain tail
                for hh in range(2):
                    nc.sync.dma_start(
                        out_d[:, st, hh * (NB // 2):(hh + 1) * (NB // 2), :],
                        o16_t[:, hh * (NB // 2):(hh + 1) * (NB // 2), :],
                    )
    nc.compile()
    return nc


def _get_nc():
    global _nc_cache
    if _nc_cache is None:
        _nc_cache = _build()
    return _nc_cache


def _prep_in_maps(inputs, clusters):
    x = np.asarray(inputs, dtype=np.float32)
    c = np.asarray(clusters, dtype=np.float32)

    xh = x.astype(np.float16)
    # [core][p, st, ch, s] = xh[core*R + st*S + s, ch*128 + p]
    xt_all = np.ascontiguousarray(
        xh.reshape(NCORES, NST, S, 2, 128).transpose(0, 4, 1, 3, 2))
    # x2 row (consistent with the fp16-rounded x), shifted by -256
    x2_all = ((xh.astype(np.float64) ** 2).sum(1) - 256.0).astype(
        np.float16).reshape(NCORES, 1, R)

    ch = c.astype(np.float16)
    c2h = (ch.astype(np.float64) ** 2).sum(1)

    ko = np.zeros((128, KW), np.float16)
    # ct: -2 * ch.T  (exact doubling in fp16)
    ko[:, KO_CT:KO_CT + 512] = np.ascontiguousarray(
        (-2.0 * ch.astype(np.float32)).astype(np.float16).T
    ).reshape(2, 128, K).transpose(1, 0, 2).reshape(128, 512)
    ko[0, KO_CAUG:KO_CAUG + K] = 1.0
    ko[1, KO_CAUG:KO_CAUG + K] = (c2h - 256.0).astype(np.float16)
    ko[2, KO_CAUG:KO_CAUG + K] = 513.0

    return [
        {"xt": xt_all[i], "x2": x2_all[i], "ko": ko}
        for i in range(NCORES)
    ]


def _run(inputs, clusters, trace=False, tmpdir=None):
    nc = _get_nc()
    in_maps = _prep_in_maps(inputs, clusters)
    res = run_bass_kernel_spmd(nc, in_maps, list(range(NCORES)),
                               trace=trace, tmpdir=tmpdir)
    # device out: [128, NST, NB, K] fp16 with q[st*1024 + b*128 + p, k]
    out = np.concatenate(
        [np.asarray(res.results[i]["out"])
         .transpose(1, 2, 0, 3).reshape(R, K) for i in range(NCORES)],
        axis=0).astype(np.float32)
    return out, res


def kernel(inputs, clusters):
    out, _ = _run(inputs, clusters, trace=False)
    return out
